# revision 37
# baseline (speedup 1.0000x reference)
"""Multi-head attention (B=2, S=2048, D=1024, H=16, Dk=64) on 8 NeuronCores.

Sharding: 2-way data parallel over batch x 4-way tensor parallel over heads.
Core c handles batch c//4 and heads (c%4)*4 .. (c%4)*4+3, i.e. a 256-column
slice of the QKV projections and the matching 256-row slice of Wo. Each core
computes a partial output projection [S, D] in bf16; the host sums the 4
partials per batch in fp32 (the all-reduce of the sharding hint) and stacks
the batches.

All matmul operands are bf16 (PE full rate + fast weight load; fp32/f32r
matmuls run in multi-pass fp32_mode=HIGH at ~1/3 rate), accumulation is
always fp32 in PSUM. x is transposed AND pre-tiled on the host into the
exact SBUF layouts ([partition, chunk, free], j-block major for x^T) so
every DMA is a contiguous per-partition slab - descriptor generation cost
on the issuing engine is negligible and the first K projection starts after
~1MB of transfer.

On-core algorithm:
  Q^T, K^T head-packed [128, 2, S] (head parity on partition halves 0-63/
  64-127 so the two heads' K=64 score matmuls run concurrently in separate
  PE row groups) and V in natural [t, d'] layout augmented with a ones
  column -> S^T = K_h Q_h^T -> exp (1/8 scale folded in; no max subtraction:
  scores are O(5)) -> C^T = V_aug^T @ expS^T where the ones row yields the
  softmax denominator for free -> normalize -> partial out = C^T.T @ Wo.

The exp stream is split between the ACT engine (exact table exp) and the
otherwise-idle DVE via a one-instruction Schraudolph exp in bf16 bit space
(scores -> affine -> int16 round -> bitcast bf16), used on a minority of
t-tiles so the added rms error stays well under the tolerance.

Emission interleaves the projections and output-projection drains into the
attention t-loop so the PE never idles (HAM stays warm) while ACT/DVE grind
through the exp stream.
"""
from collections import defaultdict
from contextlib import ExitStack

import numpy as np
import ml_dtypes
import concourse.bass as bass
import concourse.mybir as mybir
import concourse.tile as tile
from concourse import bacc
from concourse.bass_utils import run_bass_kernel_spmd
from concourse.masks import make_identity

f32 = mybir.dt.float32
bf16 = mybir.dt.bfloat16
fp8 = mybir.dt.float8e4
i16 = mybir.dt.int16
DR = mybir.MatmulPerfMode.DoubleRow
AF = mybir.ActivationFunctionType
ALU = mybir.AluOpType
np_bf16 = ml_dtypes.bfloat16

B, S, D = 2, 2048, 1024
H, DK = 16, 64
NCORES = 8
TP = 4                 # tensor-parallel factor (head groups)
HPC = H // TP          # 4 heads per core
DP = HPC * DK          # 256 = per-core d' slice
SBK = 512              # s-block for attention streaming
NSB = S // SBK         # 4
NT = S // 128          # 16 t-tiles
NDC = D // 128         # 8 contraction chunks over D
NPC = DP // 128        # 2 chunks over d'

# Schraudolph fast-exp in bf16 bit space: bits(2^g) = round(2^7*(g+127-c)),
# g = s * log2e/8; c tuned for min rms of the linear-mantissa approximation.
SCH_C = 0.0580
SCH_A = float(2.0**7 * np.log2(np.e) / 8.0)
SCH_B = float(2.0**7 * (127.0 - 4.0 - SCH_C))  # extra -4: exp values scaled
# by 1/16 (cancels in the softmax ratio) so the fp8 tiles stay in range
ESC = float(-4.0 * np.log(2.0))               # ACT bias: exp(s/8 - 4ln2)

# (block, t) tiles whose exp runs on the DVE instead of ACT; chosen as
# aligned t-pairs because ACT pairs feed DoubleRow-fp8 PV matmuls while
# DVE pairs keep plain bf16 PV.
DVE_TILES = frozenset(
    [(b, t) for b in range(2, 8) for t in (6, 7, 12, 13)]
    # extra pairs where the ACT engine (not the PE) paces the block:
    # B1 has no fillers competing, B4-B7 carry the outproj drains
    + [(1, t) for t in (6, 7, 12, 13)]
    + [(b, t) for b in range(4, 8) for t in (2, 3)]
)

_prog_cache = {}


def _build_program():
    nc = bacc.Bacc()
    # dram layouts == sbuf tile layouts (host pre-tiles): contiguous DMAs
    xt = nc.dram_tensor("xt", [128, NSB, NDC, SBK], bf16, kind="ExternalInput")
    wq = nc.dram_tensor("wq", [128, NPC, NDC, 128], bf16, kind="ExternalInput")
    wk = nc.dram_tensor("wk", [128, NPC, NDC, 128], bf16, kind="ExternalInput")
    wv = nc.dram_tensor("wv", [128, NDC, DP], bf16, kind="ExternalInput")
    wo = nc.dram_tensor("wo", [128, NPC, D], bf16, kind="ExternalInput")
    bq = nc.dram_tensor("bq", [128, NPC], f32, kind="ExternalInput")
    bk = nc.dram_tensor("bk", [128, NPC], f32, kind="ExternalInput")
    bv = nc.dram_tensor("bv", [1, DP], f32, kind="ExternalInput")
    out = nc.dram_tensor("out", [S, D], bf16, kind="ExternalOutput")

    with tile.TileContext(nc) as tc, ExitStack() as top:
        const = top.enter_context(tc.tile_pool(name="const", bufs=1))
        big = top.enter_context(tc.tile_pool(name="big", bufs=1))
        esp = top.enter_context(tc.tile_pool(name="esp", bufs=5))
        smal = top.enter_context(tc.tile_pool(name="smal", bufs=2))
        outp = top.enter_context(tc.tile_pool(name="outp", bufs=2))
        ps_s = top.enter_context(tc.tile_pool(name="ps_s", bufs=2, space="PSUM"))
        ps_x = top.enter_context(tc.tile_pool(name="ps_x", bufs=2, space="PSUM"))
        ps_c = top.enter_context(tc.tile_pool(name="ps_c", bufs=1, space="PSUM"))

        # persistent activations
        xt_r = big.tile([128, NSB, NDC, SBK], bf16)
        qt_r = big.tile([128, NPC, S], bf16)
        kt_r = big.tile([128, NPC, S], bf16)
        vaug = big.tile([128, NT, HPC, DK + 1], bf16)
        ct_r = big.tile([128, NPC, S], bf16)

        wq_r = const.tile([128, NPC, NDC, 128], bf16)
        wk_r = const.tile([128, NPC, NDC, 128], bf16)
        wv_r = const.tile([128, NDC, DP], bf16)
        wo_r = const.tile([128, NPC, D], bf16)
        bq_sb = const.tile([128, NPC], f32)
        bk_sb = const.tile([128, NPC], f32)
        bv_b = const.tile([128, DP], f32)

        # ---- loads: x^T j'-blocks on the sync HWDGE queue (the first K
        # projection starts after ~1MB), weights on the gpsimd SWDGE
        # queues (wk first: it gates the first matmul) ----
        # biases first on the scalar HWDGE queue (tiny; bv_1 must land
        # early so the gpsimd broadcast at the END of its queue never gates
        # the weight loads), then the second half of the first x^T block
        nc.scalar.dma_start(out=bk_sb, in_=bk[:, :])
        nc.scalar.dma_start(out=bq_sb, in_=bq[:, :])
        bv_1 = const.tile([1, DP], f32)
        nc.scalar.dma_start(out=bv_1, in_=bv[:, :])
        nc.sync.dma_start(out=xt_r[:, 0, 0:4], in_=xt[:, 0, 0:4])
        nc.scalar.dma_start(out=xt_r[:, 0, 4:8], in_=xt[:, 0, 4:8])
        for jp in range(1, NSB):
            nc.sync.dma_start(out=xt_r[:, jp], in_=xt[:, jp])
        # c=0 halves of wk/wq first (they gate the first matmuls), then
        # wv (first V rides the pre-attention gap), then the rest; the
        # bv broadcast (needed by V(0)'s drain, ~20us in) goes last
        nc.gpsimd.dma_start(out=wk_r[:, 0], in_=wk[:, 0])
        nc.gpsimd.dma_start(out=wq_r[:, 0], in_=wq[:, 0])
        nc.gpsimd.dma_start(out=wv_r, in_=wv[:, :, :])
        nc.gpsimd.dma_start(out=wk_r[:, 1], in_=wk[:, 1])
        nc.gpsimd.dma_start(out=wq_r[:, 1], in_=wq[:, 1])
        nc.gpsimd.dma_start(out=wo_r, in_=wo[:, :, :])
        nc.gpsimd.partition_broadcast(bv_b, bv_1)
        ones_f = const.tile([128, NT, HPC], bf16)
        nc.vector.memset(ones_f, 1.0)
        esc_b = const.tile([128, 1], f32)
        nc.vector.memset(esc_b, ESC)
        ident = const.tile([128, 128], f32)
        make_identity(nc, ident)
        ones64 = const.tile([1, 64], f32)
        nc.vector.memset(ones64, 1.0)
        nc.vector.tensor_copy(out=vaug[:, :, :, DK], in_=ones_f)

        # ---- projection helpers ----
        # Q/K projections are emitted in two 4-matmul halves at consecutive
        # steps so no single step carries a ~1.7us PE lump (ACT can only
        # buffer ~1 score tile ahead; any lump starves the exp stream).
        proj_halves = {}

        def proj_qk_half(wr, bias_sb, dst, c, j, half):
            key = (id(wr), c, j)
            if half == 0:
                proj_halves[key] = ps_x.tile(
                    [128, SBK], f32, tag="px", bufs=2,
                    name=f"pj{id(wr)%97}_{c}_{j}")
            pq = proj_halves[key]
            for k in range(4 * half, 4 * half + 4):
                nc.tensor.matmul(
                    out=pq,
                    lhsT=wr[:, c, k, :],
                    rhs=xt_r[:, j, k, :],
                    start=(k == 0), stop=(k == NDC - 1),
                )
            if half == 1:
                nc.vector.tensor_scalar_add(
                    out=dst[:, c, j * SBK:(j + 1) * SBK],
                    in0=pq, scalar1=bias_sb[:, c:c + 1],
                )

        def proj_qk(wr, bias_sb, dst, c, j):
            proj_qk_half(wr, bias_sb, dst, c, j, 0)
            proj_qk_half(wr, bias_sb, dst, c, j, 1)

        def proj_v(st):
            pv = ps_x.tile([128, DP], f32, tag="px", bufs=2, name=f"pv{st}")
            for k in range(NDC):
                nc.tensor.matmul(
                    out=pv,
                    lhsT=xt_r[:, st // 4, k, (st % 4) * 128:(st % 4 + 1) * 128],
                    rhs=wv_r[:, k, :],
                    start=(k == 0), stop=(k == NDC - 1),
                )
            nc.vector.tensor_add(
                out=vaug[:, st, :, 0:DK],
                in0=pv.rearrange("p (h d) -> p h d", h=HPC),
                in1=bv_b.rearrange("p (h d) -> p h d", h=HPC),
            )

        drain_flip = [0]

        def outproj_result(st, nh, use_ss=False):
            # tail results borrow the score-psum banks (idle after the last
            # exp) so four results pipeline instead of two
            if use_ss:
                po = ps_s.tile([128, 512], f32, tag="ss", bufs=2,
                               name=f"po{st}_{nh}")
            else:
                po = ps_x.tile([128, 512], f32, tag="px", bufs=2,
                               name=f"po{st}_{nh}")
            for c in range(NPC):
                nc.tensor.matmul(
                    out=po,
                    lhsT=ct_r[:, c, st * 128:(st + 1) * 128],
                    rhs=wo_r[:, c, nh * 512:(nh + 1) * 512],
                    start=(c == 0), stop=(c == NPC - 1),
                )
            ob = outp.tile([128, 512], bf16, tag="ob", name=f"ob{st}_{nh}")
            drain_flip[0] ^= 1
            if drain_flip[0]:
                nc.vector.tensor_copy(out=ob, in_=po)
            else:
                nc.scalar.copy(out=ob, in_=po)
            nc.sync.dma_start(
                out=out[st * 128:(st + 1) * 128, nh * 512:(nh + 1) * 512],
                in_=ob,
            )

        # normalize runs in three stages emitted ~2 steps apart so no
        # DVE/gpsimd op is ever enqueued before its cross-engine dependency
        # is already satisfied (head-of-line blocking stalled the whole
        # pipeline at every block boundary otherwise).
        norm_state = {}

        def norm_stage1(j, hp, pcs):
            cus = []
            for hh in range(2):
                cu = smal.tile([DK + 1, SBK], f32, tag=f"cu{hh}", bufs=1,
                               name=f"cu{j}{hp}{hh}")
                if hh == 0:
                    nc.scalar.copy(out=cu, in_=pcs[hh])
                else:
                    nc.vector.tensor_copy(out=cu, in_=pcs[hh])
                cus.append(cu)
            # denominator rows [1,512] -> partition-major [128,8] via eight
            # skinny PE transposes (the gpsimd gather DMA costs ~4us in
            # 4-byte descriptors; the PE does it in ~1us)
            dn_ps = ps_x.tile([128, 8], f32, tag="px", bufs=2,
                              name=f"dnp{j}{hp}")
            for hh in range(2):
                for cc in range(4):
                    nc.tensor.transpose(
                        out=dn_ps[:, hh * 4 + cc:hh * 4 + cc + 1],
                        in_=cus[hh][DK:DK + 1, cc * 128:(cc + 1) * 128],
                        identity=ident[DK:DK + 1, DK:DK + 1],
                    )
            dnT = smal.tile([128, 8], f32, tag="dnT", name=f"dnT{j}{hp}")
            nc.vector.tensor_copy(out=dnT, in_=dn_ps)
            norm_state[(j, hp)] = [cus, dnT, None]

        def norm_stage2(j, hp):
            cus, dnT, _ = norm_state[(j, hp)]
            rT = smal.tile([128, 8], f32, tag="rT", name=f"rT{j}{hp}")
            nc.vector.reciprocal(out=rT, in_=dnT)
            # back to partition-0 rows (rhs of a matmul must be 0/32/64/96
            # aligned) via skinny transposes, then broadcast down 64
            # partitions per head half with ones-column K=1 matmuls
            rf = smal.tile([1, 2, SBK], f32, tag="rf", name=f"rf{j}{hp}")
            for hh in range(2):
                rfp = ps_x.tile([1, SBK], f32, tag="px", bufs=2,
                                name=f"rfp{j}{hp}{hh}")
                for cc in range(4):
                    nc.tensor.transpose(
                        out=rfp[0:1, cc * 128:(cc + 1) * 128],
                        in_=rT[:, hh * 4 + cc:hh * 4 + cc + 1],
                        identity=ident,
                    )
                nc.vector.tensor_copy(out=rf[:, hh, :], in_=rfp)
            rb_ps = ps_x.tile([128, SBK], f32, tag="px", bufs=2,
                              name=f"rbp{j}{hp}")
            for hh in range(2):
                for cc in range(4):
                    nc.tensor.matmul(
                        out=rb_ps[hh * 64:(hh + 1) * 64,
                                  cc * 128:(cc + 1) * 128],
                        lhsT=ones64,
                        rhs=rf[0:1, hh, cc * 128:(cc + 1) * 128],
                        start=True, stop=True,
                        tile_position=(0, hh * 64),
                    )
            norm_state[(j, hp)][2] = rb_ps

        def norm_stage3(j, hp):
            cus, _, rb_ps = norm_state[(j, hp)]
            for hh in range(2):
                nc.vector.tensor_mul(
                    out=ct_r[hh * 64:(hh + 1) * 64, hp, j * SBK:(j + 1) * SBK],
                    in0=cus[hh][0:DK, :],
                    in1=rb_ps[hh * 64:(hh + 1) * 64, :],
                )

        TAIL_OPS = []

        # ---- static filler schedule: (block, t) -> list of closures ----
        # MAND runs before the step's score matmul (front=True prepends:
        # projection halves must free their psum bank before V/outproj
        # allocate it); POST runs after the step's exp is emitted.
        # Blocks run hp-major: B0..B3 = (j,0), B4..B7 = (j,1). The c=1
        # projections aren't needed until B4, so the early blocks stay light
        # and the exp stream paces the kernel from B1 on.
        BLOCKS = [(j, 0) for j in range(NSB)] + [(j, 1) for j in range(NSB)]
        MAND = defaultdict(list)

        POST = defaultdict(list)

        def sched(b, t, fn, front=False):
            if front:
                MAND[(b, t)].insert(0, fn)
            else:
                MAND[(b, t)].append(fn)

        def sched_proj(b, t0, wr, bias_sb, dst, c, j):
            sched(b, t0, (lambda: proj_qk_half(wr, bias_sb, dst, c, j, 0)),
                  front=True)
            sched(b, t0 + 1, (lambda: proj_qk_half(wr, bias_sb, dst, c, j, 1)),
                  front=True)

        # B0 ((j0,hp0)): V projections ride post-step (V(st) emitted right
        # after step st's exp, consumed by the PV pair drained entering step
        # st+3; never ahead of the score matmuls, so the exp stream starts
        # as soon as the first K/Q chunks land), remaining K(c0) chunks land
        # just before their t-tiles need them.
        for st in range(NT):
            POST[(0, st)].append((lambda st=st: proj_v(st)))
        sched_proj(0, 1, wk_r, bk_sb, kt_r, 0, 1)
        sched_proj(0, 4, wk_r, bk_sb, kt_r, 0, 2)
        sched_proj(0, 8, wk_r, bk_sb, kt_r, 0, 3)
        sched_proj(0, 11, wq_r, bq_sb, qt_r, 0, 1)
        # B1..B3: spread K(c1) and the remaining Q projections evenly
        sched_proj(1, 4, wk_r, bk_sb, kt_r, 1, 0)
        sched_proj(1, 11, wq_r, bq_sb, qt_r, 0, 2)
        sched_proj(2, 1, wk_r, bk_sb, kt_r, 1, 1)
        sched_proj(2, 4, wk_r, bk_sb, kt_r, 1, 2)
        sched_proj(2, 11, wq_r, bq_sb, qt_r, 0, 3)
        sched_proj(3, 1, wk_r, bk_sb, kt_r, 1, 3)
        sched_proj(3, 4, wq_r, bq_sb, qt_r, 1, 0)
        sched_proj(3, 11, wq_r, bq_sb, qt_r, 1, 1)
        sched_proj(4, 8, wq_r, bq_sb, qt_r, 1, 2)
        sched_proj(5, 8, wq_r, bq_sb, qt_r, 1, 3)
        # normalize stages 2/3 of block b land early in block b+1
        for b in range(2 * NSB - 1):
            j, hp = BLOCKS[b]
            sched(b + 1, 5, (lambda j=j, hp=hp: norm_stage2(j, hp)))
            sched(b + 1, 6, (lambda j=j, hp=hp: norm_stage3(j, hp)))
        # output projection of j needs ct from (j,0) AND (j,1); block (j,1)
        # is B4+j, its normalize finishes early in B5+j -> spread the eight
        # [128x512] results over B5+j / B6+j (j=2 spills 2, j=3 fully into
        # the tail).
        OP_SLOTS = [(0, 7), (0, 9), (0, 11), (0, 13), (0, 15),
                    (1, 1), (1, 3), (1, 5)]
        OP_SLOTS_LAST = [(0, 7), (0, 8), (0, 9), (0, 10),
                         (0, 11), (0, 12), (0, 13), (0, 14)]
        for j in (0, 1, 2):
            slots = OP_SLOTS_LAST if j == 2 else OP_SLOTS
            for i, (st, nh) in enumerate(
                    (st, nh)
                    for st in range(j * 4, (j + 1) * 4) for nh in range(2)):
                db, tt = slots[i]
                if 5 + j + db < 2 * NSB:
                    sched(5 + j + db, tt,
                          (lambda st=st, nh=nh: outproj_result(st, nh)))
                else:
                    TAIL_OPS.append((st, nh))

        # ---- attention driver: software-pipelined, drains t-pairs ----
        pend = []
        cur_es8 = [None]

        def drain_pv():
            j, hp, t0, es0, pcs = pend.pop(0)
            _, _, t1, es1, _ = pend.pop(0)
            for t, es in ((t0, es0), (t1, es1)):
                for hh in range(2):
                    nc.tensor.matmul(
                        out=pcs[hh],
                        lhsT=vaug[:, t, hp * 2 + hh, :],
                        rhs=es[:, hh, :],
                        start=(t == 0), stop=(t == NT - 1),
                    )
            if t1 == NT - 1:
                norm_stage1(j, hp, pcs)

        # pre-attention: K/Q for the first score matmul only
        proj_qk(wk_r, bk_sb, kt_r, 0, 0)
        proj_qk(wq_r, bq_sb, qt_r, 0, 0)

        pcs_by = {}
        for b in range(2 * NSB):
            j, hp = BLOCKS[b]
            pcs_by[(j, hp)] = [
                ps_c.tile([DK + 1, SBK], f32, tag=f"pc{hh}", name=f"pc{hh}_{j}_{hp}")
                for hh in range(2)]
            for t in range(NT):
                if len(pend) >= 4 or (t == 1 and len(pend) >= 2):
                    drain_pv()
                for fn in MAND[(b, t)]:
                    fn()
                ss = ps_s.tile([128, 2, SBK], f32, tag="ss", name=f"ss{b}_{t}")
                for hh in range(2):
                    nc.tensor.matmul(
                        out=ss[:, hh, :],
                        lhsT=kt_r[hh * 64:(hh + 1) * 64, hp, t * 128:(t + 1) * 128],
                        rhs=qt_r[hh * 64:(hh + 1) * 64, hp, j * SBK:(j + 1) * SBK],
                        start=True, stop=True,
                    )
                if (b, t) in DVE_TILES:
                    # Schraudolph: bf16 bits of exp(s/8)/16 = int16(A*s + B);
                    # the PV matmul reads the int16 tile as bf16 directly.
                    esi = esp.tile([128, 2, SBK], i16, tag="es", name=f"esi{b}_{t}")
                    nc.vector.tensor_scalar(
                        out=esi, in0=ss,
                        scalar1=SCH_A, scalar2=SCH_B,
                        op0=ALU.mult, op1=ALU.add,
                    )
                    es = esi.bitcast(bf16)
                else:
                    es = esp.tile([128, 2, SBK], bf16, tag="es", name=f"es{b}_{t}")
                    nc.scalar.activation(out=es, in_=ss, func=AF.Exp,
                                         scale=0.125, bias=esc_b[:, 0:1])
                pend.append((j, hp, t, es, pcs_by[(j, hp)]))
                for fn in POST[(b, t)]:
                    fn()
        drain_pv()
        drain_pv()
        # tail: finish the last block's normalize, then the remaining
        # output-projection results
        norm_stage2(3, 1)
        norm_stage3(3, 1)
        for st in range(12, 16):
            for nh in range(2):
                TAIL_OPS.append((st, nh))
        for i, (st, nh) in enumerate(TAIL_OPS):
            outproj_result(st, nh, use_ss=(i % 2 == 1))

    nc.finalize()
    return nc


def _get_program():
    if "nc" not in _prog_cache:
        _prog_cache["nc"] = _build_program()
    return _prog_cache["nc"]


def _pretile_k(w):
    """[D, d'] fp32 -> [128, D//128, d'] bf16 (partition-major chunks)."""
    dp = w.shape[1]
    return np.ascontiguousarray(
        w.reshape(-1, 128, dp).transpose(1, 0, 2).astype(np_bf16))


def _pretile_qk(w):
    """[D, 256] fp32 -> [128, 2, D//128, 128] bf16 (c-major halves)."""
    return np.ascontiguousarray(
        w.reshape(NDC, 128, NPC, 128).transpose(1, 2, 0, 3).astype(np_bf16))


def _make_in_maps(x, Wq, bq, Wk, bk, Wv, bv, Wo, bo):
    # x^T pre-tiled: xt[p, jp, k, s'] = x[jp*512+s', k*128+p]
    xts = []
    for b in range(B):
        xt = x[b].T.reshape(NDC, 128, NSB, SBK).transpose(1, 2, 0, 3)
        xts.append(np.ascontiguousarray(xt.astype(np_bf16)))
    in_maps = []
    for c in range(NCORES):
        b, hg = divmod(c, TP)
        sl = slice(hg * DP, (hg + 1) * DP)
        in_maps.append({
            "xt": xts[b],
            "wq": _pretile_qk(Wq[:, sl]),
            "wk": _pretile_qk(Wk[:, sl]),
            "wv": _pretile_k(Wv[:, sl]),
            "wo": _pretile_k(Wo[sl, :]),
            "bq": np.ascontiguousarray(bq[sl].reshape(NPC, 128).T),
            "bk": np.ascontiguousarray(bk[sl].reshape(NPC, 128).T),
            "bv": np.ascontiguousarray(bv[sl].reshape(1, DP)),
        })
    return in_maps


def run(inputs, **spmd_kwargs):
    """Build, run on 8 cores, gather. Returns (output, BassKernelResults)."""
    args = {k: np.asarray(v, dtype=np.float32) for k, v in inputs.items()}
    nc = _get_program()
    in_maps = _make_in_maps(
        args["x"], args["Wq"], args["bq"], args["Wk"], args["bk"],
        args["Wv"], args["bv"], args["Wo"], args["bo"],
    )
    res = run_bass_kernel_spmd(nc, in_maps, list(range(NCORES)), **spmd_kwargs)
    out = np.zeros((B, S, D), dtype=np.float32)
    for c in range(NCORES):
        b = c // TP
        out[b] += res.results[c]["out"].astype(np.float32)
    out += args["bo"]
    return out, res


def kernel(**inputs):
    out, _ = run(inputs)
    return out


# revision 38
# speedup vs baseline: 1.0029x; 1.0029x over previous
"""Multi-head attention (B=2, S=2048, D=1024, H=16, Dk=64) on 8 NeuronCores.

Sharding: 2-way data parallel over batch x 4-way tensor parallel over heads.
Core c handles batch c//4 and heads (c%4)*4 .. (c%4)*4+3, i.e. a 256-column
slice of the QKV projections and the matching 256-row slice of Wo. Each core
computes a partial output projection [S, D] in bf16; the host sums the 4
partials per batch in fp32 (the all-reduce of the sharding hint) and stacks
the batches.

All matmul operands are bf16 (PE full rate + fast weight load; fp32/f32r
matmuls run in multi-pass fp32_mode=HIGH at ~1/3 rate), accumulation is
always fp32 in PSUM. x is transposed AND pre-tiled on the host into the
exact SBUF layouts ([partition, chunk, free], j-block major for x^T) so
every DMA is a contiguous per-partition slab - descriptor generation cost
on the issuing engine is negligible and the first K projection starts after
~1MB of transfer.

On-core algorithm:
  Q^T, K^T head-packed [128, 2, S] (head parity on partition halves 0-63/
  64-127 so the two heads' K=64 score matmuls run concurrently in separate
  PE row groups) and V in natural [t, d'] layout augmented with a ones
  column -> S^T = K_h Q_h^T -> exp (1/8 scale folded in; no max subtraction:
  scores are O(5)) -> C^T = V_aug^T @ expS^T where the ones row yields the
  softmax denominator for free -> normalize -> partial out = C^T.T @ Wo.

The exp stream is split between the ACT engine (exact table exp) and the
otherwise-idle DVE via a one-instruction Schraudolph exp in bf16 bit space
(scores -> affine -> int16 round -> bitcast bf16), used on a minority of
t-tiles so the added rms error stays well under the tolerance.

Emission interleaves the projections and output-projection drains into the
attention t-loop so the PE never idles (HAM stays warm) while ACT/DVE grind
through the exp stream.
"""
from collections import defaultdict
from contextlib import ExitStack

import numpy as np
import ml_dtypes
import concourse.bass as bass
import concourse.mybir as mybir
import concourse.tile as tile
from concourse import bacc
from concourse.bass_utils import run_bass_kernel_spmd
from concourse.masks import make_identity

f32 = mybir.dt.float32
bf16 = mybir.dt.bfloat16
fp8 = mybir.dt.float8e4
i16 = mybir.dt.int16
DR = mybir.MatmulPerfMode.DoubleRow
AF = mybir.ActivationFunctionType
ALU = mybir.AluOpType
np_bf16 = ml_dtypes.bfloat16

B, S, D = 2, 2048, 1024
H, DK = 16, 64
NCORES = 8
TP = 4                 # tensor-parallel factor (head groups)
HPC = H // TP          # 4 heads per core
DP = HPC * DK          # 256 = per-core d' slice
SBK = 512              # s-block for attention streaming
NSB = S // SBK         # 4
NT = S // 128          # 16 t-tiles
NDC = D // 128         # 8 contraction chunks over D
NPC = DP // 128        # 2 chunks over d'

# Schraudolph fast-exp in bf16 bit space: bits(2^g) = round(2^7*(g+127-c)),
# g = s * log2e/8; c tuned for min rms of the linear-mantissa approximation.
SCH_C = 0.0580
SCH_A = float(2.0**7 * np.log2(np.e) / 8.0)
SCH_B = float(2.0**7 * (127.0 - 4.0 - SCH_C))  # extra -4: exp values scaled
# by 1/16 (cancels in the softmax ratio) so the fp8 tiles stay in range
ESC = float(-4.0 * np.log(2.0))               # ACT bias: exp(s/8 - 4ln2)

# (block, t) tiles whose exp runs on the DVE instead of ACT; chosen as
# aligned t-pairs because ACT pairs feed DoubleRow-fp8 PV matmuls while
# DVE pairs keep plain bf16 PV.
DVE_TILES = frozenset(
    (b, t) for b in range(2, 8) for t in (6, 7, 12, 13)
)

_prog_cache = {}


def _build_program():
    nc = bacc.Bacc()
    # dram layouts == sbuf tile layouts (host pre-tiles): contiguous DMAs
    xt = nc.dram_tensor("xt", [128, NSB, NDC, SBK], bf16, kind="ExternalInput")
    wq = nc.dram_tensor("wq", [128, NPC, NDC, 128], bf16, kind="ExternalInput")
    wk = nc.dram_tensor("wk", [128, NPC, NDC, 128], bf16, kind="ExternalInput")
    wv = nc.dram_tensor("wv", [128, NDC, DP], bf16, kind="ExternalInput")
    wo = nc.dram_tensor("wo", [128, NPC, D], bf16, kind="ExternalInput")
    bq = nc.dram_tensor("bq", [128, NPC], f32, kind="ExternalInput")
    bk = nc.dram_tensor("bk", [128, NPC], f32, kind="ExternalInput")
    bv = nc.dram_tensor("bv", [1, DP], f32, kind="ExternalInput")
    out = nc.dram_tensor("out", [S, D], bf16, kind="ExternalOutput")

    with tile.TileContext(nc) as tc, ExitStack() as top:
        const = top.enter_context(tc.tile_pool(name="const", bufs=1))
        big = top.enter_context(tc.tile_pool(name="big", bufs=1))
        esp = top.enter_context(tc.tile_pool(name="esp", bufs=5))
        smal = top.enter_context(tc.tile_pool(name="smal", bufs=2))
        outp = top.enter_context(tc.tile_pool(name="outp", bufs=2))
        ps_s = top.enter_context(tc.tile_pool(name="ps_s", bufs=2, space="PSUM"))
        ps_x = top.enter_context(tc.tile_pool(name="ps_x", bufs=2, space="PSUM"))
        ps_c = top.enter_context(tc.tile_pool(name="ps_c", bufs=1, space="PSUM"))

        # persistent activations
        xt_r = big.tile([128, NSB, NDC, SBK], bf16)
        qt_r = big.tile([128, NPC, S], bf16)
        kt_r = big.tile([128, NPC, S], bf16)
        vaug = big.tile([128, NT, HPC, DK + 1], bf16)
        ct_r = big.tile([128, NPC, S], bf16)

        wq_r = const.tile([128, NPC, NDC, 128], bf16)
        wk_r = const.tile([128, NPC, NDC, 128], bf16)
        wv_r = const.tile([128, NDC, DP], bf16)
        wo_r = const.tile([128, NPC, D], bf16)
        bq_sb = const.tile([128, NPC], f32)
        bk_sb = const.tile([128, NPC], f32)
        bv_b = const.tile([128, DP], f32)

        # ---- loads: x^T j'-blocks on the sync HWDGE queue (the first K
        # projection starts after ~1MB), weights on the gpsimd SWDGE
        # queues (wk first: it gates the first matmul) ----
        # biases first on the scalar HWDGE queue (tiny; bv_1 must land
        # early so the gpsimd broadcast at the END of its queue never gates
        # the weight loads), then the second half of the first x^T block
        nc.scalar.dma_start(out=bk_sb, in_=bk[:, :])
        nc.scalar.dma_start(out=bq_sb, in_=bq[:, :])
        bv_1 = const.tile([1, DP], f32)
        nc.scalar.dma_start(out=bv_1, in_=bv[:, :])
        nc.sync.dma_start(out=xt_r[:, 0, 0:4], in_=xt[:, 0, 0:4])
        nc.scalar.dma_start(out=xt_r[:, 0, 4:8], in_=xt[:, 0, 4:8])
        for jp in range(1, NSB):
            nc.sync.dma_start(out=xt_r[:, jp], in_=xt[:, jp])
        # c=0 halves of wk/wq first (they gate the first matmuls), then
        # wv (first V rides the pre-attention gap), then the rest; the
        # bv broadcast (needed by V(0)'s drain, ~20us in) goes last
        nc.gpsimd.dma_start(out=wk_r[:, 0], in_=wk[:, 0])
        nc.gpsimd.dma_start(out=wq_r[:, 0], in_=wq[:, 0])
        nc.gpsimd.dma_start(out=wv_r, in_=wv[:, :, :])
        nc.gpsimd.dma_start(out=wk_r[:, 1], in_=wk[:, 1])
        nc.gpsimd.dma_start(out=wq_r[:, 1], in_=wq[:, 1])
        nc.gpsimd.dma_start(out=wo_r, in_=wo[:, :, :])
        nc.gpsimd.partition_broadcast(bv_b, bv_1)
        ones_f = const.tile([128, NT, HPC], bf16)
        nc.vector.memset(ones_f, 1.0)
        esc_b = const.tile([128, 1], f32)
        nc.vector.memset(esc_b, ESC)
        ident = const.tile([128, 128], f32)
        make_identity(nc, ident)
        ones64 = const.tile([1, 64], f32)
        nc.vector.memset(ones64, 1.0)
        nc.vector.tensor_copy(out=vaug[:, :, :, DK], in_=ones_f)

        # ---- projection helpers ----
        # Q/K projections are emitted in two 4-matmul halves at consecutive
        # steps so no single step carries a ~1.7us PE lump (ACT can only
        # buffer ~1 score tile ahead; any lump starves the exp stream).
        proj_halves = {}

        def proj_qk_half(wr, bias_sb, dst, c, j, half):
            key = (id(wr), c, j)
            if half == 0:
                proj_halves[key] = ps_x.tile(
                    [128, SBK], f32, tag="px", bufs=2,
                    name=f"pj{id(wr)%97}_{c}_{j}")
            pq = proj_halves[key]
            for k in range(4 * half, 4 * half + 4):
                nc.tensor.matmul(
                    out=pq,
                    lhsT=wr[:, c, k, :],
                    rhs=xt_r[:, j, k, :],
                    start=(k == 0), stop=(k == NDC - 1),
                )
            if half == 1:
                nc.vector.tensor_scalar_add(
                    out=dst[:, c, j * SBK:(j + 1) * SBK],
                    in0=pq, scalar1=bias_sb[:, c:c + 1],
                )

        def proj_qk(wr, bias_sb, dst, c, j):
            proj_qk_half(wr, bias_sb, dst, c, j, 0)
            proj_qk_half(wr, bias_sb, dst, c, j, 1)

        def proj_v(st):
            pv = ps_x.tile([128, DP], f32, tag="px", bufs=2, name=f"pv{st}")
            for k in range(NDC):
                nc.tensor.matmul(
                    out=pv,
                    lhsT=xt_r[:, st // 4, k, (st % 4) * 128:(st % 4 + 1) * 128],
                    rhs=wv_r[:, k, :],
                    start=(k == 0), stop=(k == NDC - 1),
                )
            nc.vector.tensor_add(
                out=vaug[:, st, :, 0:DK],
                in0=pv.rearrange("p (h d) -> p h d", h=HPC),
                in1=bv_b.rearrange("p (h d) -> p h d", h=HPC),
            )

        drain_flip = [0]

        def outproj_result(st, nh, use_ss=False):
            # tail results borrow the score-psum banks (idle after the last
            # exp) so four results pipeline instead of two
            if use_ss:
                po = ps_s.tile([128, 512], f32, tag="ss", bufs=2,
                               name=f"po{st}_{nh}")
            else:
                po = ps_x.tile([128, 512], f32, tag="px", bufs=2,
                               name=f"po{st}_{nh}")
            for c in range(NPC):
                nc.tensor.matmul(
                    out=po,
                    lhsT=ct_r[:, c, st * 128:(st + 1) * 128],
                    rhs=wo_r[:, c, nh * 512:(nh + 1) * 512],
                    start=(c == 0), stop=(c == NPC - 1),
                )
            ob = outp.tile([128, 512], bf16, tag="ob", name=f"ob{st}_{nh}")
            drain_flip[0] ^= 1
            if drain_flip[0]:
                nc.vector.tensor_copy(out=ob, in_=po)
            else:
                nc.scalar.copy(out=ob, in_=po)
            nc.sync.dma_start(
                out=out[st * 128:(st + 1) * 128, nh * 512:(nh + 1) * 512],
                in_=ob,
            )

        # normalize runs in three stages emitted ~2 steps apart so no
        # DVE/gpsimd op is ever enqueued before its cross-engine dependency
        # is already satisfied (head-of-line blocking stalled the whole
        # pipeline at every block boundary otherwise).
        norm_state = {}

        def norm_stage1(j, hp, pcs):
            cus = []
            for hh in range(2):
                cu = smal.tile([DK + 1, SBK], f32, tag=f"cu{hh}", bufs=1,
                               name=f"cu{j}{hp}{hh}")
                if hh == 0:
                    nc.scalar.copy(out=cu, in_=pcs[hh])
                else:
                    nc.vector.tensor_copy(out=cu, in_=pcs[hh])
                cus.append(cu)
            # denominator rows [1,512] -> partition-major [128,8] via eight
            # skinny PE transposes (the gpsimd gather DMA costs ~4us in
            # 4-byte descriptors; the PE does it in ~1us)
            dn_ps = ps_x.tile([128, 8], f32, tag="px", bufs=2,
                              name=f"dnp{j}{hp}")
            for hh in range(2):
                for cc in range(4):
                    nc.tensor.transpose(
                        out=dn_ps[:, hh * 4 + cc:hh * 4 + cc + 1],
                        in_=cus[hh][DK:DK + 1, cc * 128:(cc + 1) * 128],
                        identity=ident[DK:DK + 1, DK:DK + 1],
                    )
            dnT = smal.tile([128, 8], f32, tag="dnT", name=f"dnT{j}{hp}")
            nc.vector.tensor_copy(out=dnT, in_=dn_ps)
            norm_state[(j, hp)] = [cus, dnT, None]

        def norm_stage2(j, hp):
            cus, dnT, _ = norm_state[(j, hp)]
            rT = smal.tile([128, 8], f32, tag="rT", name=f"rT{j}{hp}")
            nc.vector.reciprocal(out=rT, in_=dnT)
            # back to partition-0 rows (rhs of a matmul must be 0/32/64/96
            # aligned) via skinny transposes, then broadcast down 64
            # partitions per head half with ones-column K=1 matmuls
            rf = smal.tile([1, 2, SBK], f32, tag="rf", name=f"rf{j}{hp}")
            for hh in range(2):
                rfp = ps_x.tile([1, SBK], f32, tag="px", bufs=2,
                                name=f"rfp{j}{hp}{hh}")
                for cc in range(4):
                    nc.tensor.transpose(
                        out=rfp[0:1, cc * 128:(cc + 1) * 128],
                        in_=rT[:, hh * 4 + cc:hh * 4 + cc + 1],
                        identity=ident,
                    )
                nc.vector.tensor_copy(out=rf[:, hh, :], in_=rfp)
            rb_ps = ps_x.tile([128, SBK], f32, tag="px", bufs=2,
                              name=f"rbp{j}{hp}")
            for hh in range(2):
                for cc in range(4):
                    nc.tensor.matmul(
                        out=rb_ps[hh * 64:(hh + 1) * 64,
                                  cc * 128:(cc + 1) * 128],
                        lhsT=ones64,
                        rhs=rf[0:1, hh, cc * 128:(cc + 1) * 128],
                        start=True, stop=True,
                        tile_position=(0, hh * 64),
                    )
            norm_state[(j, hp)][2] = rb_ps

        def norm_stage3(j, hp):
            cus, _, rb_ps = norm_state[(j, hp)]
            for hh in range(2):
                nc.vector.tensor_mul(
                    out=ct_r[hh * 64:(hh + 1) * 64, hp, j * SBK:(j + 1) * SBK],
                    in0=cus[hh][0:DK, :],
                    in1=rb_ps[hh * 64:(hh + 1) * 64, :],
                )

        TAIL_OPS = []

        # ---- static filler schedule: (block, t) -> list of closures ----
        # MAND runs before the step's score matmul (front=True prepends:
        # projection halves must free their psum bank before V/outproj
        # allocate it); POST runs after the step's exp is emitted.
        # Blocks run hp-major: B0..B3 = (j,0), B4..B7 = (j,1). The c=1
        # projections aren't needed until B4, so the early blocks stay light
        # and the exp stream paces the kernel from B1 on.
        BLOCKS = [(j, 0) for j in range(NSB)] + [(j, 1) for j in range(NSB)]
        MAND = defaultdict(list)

        POST = defaultdict(list)

        def sched(b, t, fn, front=False):
            if front:
                MAND[(b, t)].insert(0, fn)
            else:
                MAND[(b, t)].append(fn)

        def sched_proj(b, t0, wr, bias_sb, dst, c, j):
            sched(b, t0, (lambda: proj_qk_half(wr, bias_sb, dst, c, j, 0)),
                  front=True)
            sched(b, t0 + 1, (lambda: proj_qk_half(wr, bias_sb, dst, c, j, 1)),
                  front=True)

        # B0 ((j0,hp0)): V projections ride post-step (V(st) emitted right
        # after step st's exp, consumed by the PV pair drained entering step
        # st+3; never ahead of the score matmuls, so the exp stream starts
        # as soon as the first K/Q chunks land), remaining K(c0) chunks land
        # just before their t-tiles need them.
        for st in range(NT):
            POST[(0, st)].append((lambda st=st: proj_v(st)))
        sched_proj(0, 1, wk_r, bk_sb, kt_r, 0, 1)
        sched_proj(0, 4, wk_r, bk_sb, kt_r, 0, 2)
        sched_proj(0, 8, wk_r, bk_sb, kt_r, 0, 3)
        sched_proj(0, 11, wq_r, bq_sb, qt_r, 0, 1)
        # B1..B3: spread K(c1) and the remaining Q projections evenly
        sched_proj(1, 4, wk_r, bk_sb, kt_r, 1, 0)
        sched_proj(1, 11, wq_r, bq_sb, qt_r, 0, 2)
        sched_proj(2, 1, wk_r, bk_sb, kt_r, 1, 1)
        sched_proj(2, 4, wk_r, bk_sb, kt_r, 1, 2)
        sched_proj(2, 11, wq_r, bq_sb, qt_r, 0, 3)
        sched_proj(3, 1, wk_r, bk_sb, kt_r, 1, 3)
        sched_proj(3, 4, wq_r, bq_sb, qt_r, 1, 0)
        sched_proj(3, 11, wq_r, bq_sb, qt_r, 1, 1)
        sched_proj(4, 8, wq_r, bq_sb, qt_r, 1, 2)
        sched_proj(5, 8, wq_r, bq_sb, qt_r, 1, 3)
        # normalize stages 2/3 of block b land early in block b+1
        for b in range(2 * NSB - 1):
            j, hp = BLOCKS[b]
            sched(b + 1, 5, (lambda j=j, hp=hp: norm_stage2(j, hp)))
            sched(b + 1, 6, (lambda j=j, hp=hp: norm_stage3(j, hp)))
        # output projection of j needs ct from (j,0) AND (j,1); block (j,1)
        # is B4+j, its normalize finishes early in B5+j -> spread the eight
        # [128x512] results over B5+j / B6+j (j=2 spills 2, j=3 fully into
        # the tail).
        OP_SLOTS = [(0, 7), (0, 9), (0, 11), (0, 13), (0, 15),
                    (1, 1), (1, 3), (1, 5)]
        OP_SLOTS_LAST = [(0, 7), (0, 8), (0, 9), (0, 10),
                         (0, 11), (0, 12), (0, 13), (0, 14)]
        for j in (0, 1, 2):
            slots = OP_SLOTS_LAST if j == 2 else OP_SLOTS
            for i, (st, nh) in enumerate(
                    (st, nh)
                    for st in range(j * 4, (j + 1) * 4) for nh in range(2)):
                db, tt = slots[i]
                if 5 + j + db < 2 * NSB:
                    sched(5 + j + db, tt,
                          (lambda st=st, nh=nh: outproj_result(st, nh)))
                else:
                    TAIL_OPS.append((st, nh))

        # ---- attention driver: software-pipelined, drains t-pairs ----
        pend = []
        cur_es8 = [None]

        def drain_pv():
            j, hp, t0, es0, pcs = pend.pop(0)
            _, _, t1, es1, _ = pend.pop(0)
            for t, es in ((t0, es0), (t1, es1)):
                for hh in range(2):
                    nc.tensor.matmul(
                        out=pcs[hh],
                        lhsT=vaug[:, t, hp * 2 + hh, :],
                        rhs=es[:, hh, :],
                        start=(t == 0), stop=(t == NT - 1),
                    )
            if t1 == NT - 1:
                norm_stage1(j, hp, pcs)

        # pre-attention: K/Q for the first score matmul only
        proj_qk(wk_r, bk_sb, kt_r, 0, 0)
        proj_qk(wq_r, bq_sb, qt_r, 0, 0)

        pcs_by = {}
        for b in range(2 * NSB):
            j, hp = BLOCKS[b]
            pcs_by[(j, hp)] = [
                ps_c.tile([DK + 1, SBK], f32, tag=f"pc{hh}", name=f"pc{hh}_{j}_{hp}")
                for hh in range(2)]
            for t in range(NT):
                if len(pend) >= 4 or (t == 1 and len(pend) >= 2):
                    drain_pv()
                for fn in MAND[(b, t)]:
                    fn()
                ss = ps_s.tile([128, 2, SBK], f32, tag="ss", name=f"ss{b}_{t}")
                for hh in range(2):
                    nc.tensor.matmul(
                        out=ss[:, hh, :],
                        lhsT=kt_r[hh * 64:(hh + 1) * 64, hp, t * 128:(t + 1) * 128],
                        rhs=qt_r[hh * 64:(hh + 1) * 64, hp, j * SBK:(j + 1) * SBK],
                        start=True, stop=True,
                    )
                if (b, t) in DVE_TILES:
                    # Schraudolph: bf16 bits of exp(s/8)/16 = int16(A*s + B);
                    # the PV matmul reads the int16 tile as bf16 directly.
                    esi = esp.tile([128, 2, SBK], i16, tag="es", name=f"esi{b}_{t}")
                    nc.vector.tensor_scalar(
                        out=esi, in0=ss,
                        scalar1=SCH_A, scalar2=SCH_B,
                        op0=ALU.mult, op1=ALU.add,
                    )
                    es = esi.bitcast(bf16)
                else:
                    es = esp.tile([128, 2, SBK], bf16, tag="es", name=f"es{b}_{t}")
                    nc.scalar.activation(out=es, in_=ss, func=AF.Exp,
                                         scale=0.125, bias=esc_b[:, 0:1])
                pend.append((j, hp, t, es, pcs_by[(j, hp)]))
                for fn in POST[(b, t)]:
                    fn()
        drain_pv()
        drain_pv()
        # tail: finish the last block's normalize, then the remaining
        # output-projection results
        norm_stage2(3, 1)
        norm_stage3(3, 1)
        for st in range(12, 16):
            for nh in range(2):
                TAIL_OPS.append((st, nh))
        for i, (st, nh) in enumerate(TAIL_OPS):
            outproj_result(st, nh, use_ss=(i % 2 == 1))

    nc.finalize()
    return nc


def _get_program():
    if "nc" not in _prog_cache:
        _prog_cache["nc"] = _build_program()
    return _prog_cache["nc"]


def _pretile_k(w):
    """[D, d'] fp32 -> [128, D//128, d'] bf16 (partition-major chunks)."""
    dp = w.shape[1]
    return np.ascontiguousarray(
        w.reshape(-1, 128, dp).transpose(1, 0, 2).astype(np_bf16))


def _pretile_qk(w):
    """[D, 256] fp32 -> [128, 2, D//128, 128] bf16 (c-major halves)."""
    return np.ascontiguousarray(
        w.reshape(NDC, 128, NPC, 128).transpose(1, 2, 0, 3).astype(np_bf16))


def _make_in_maps(x, Wq, bq, Wk, bk, Wv, bv, Wo, bo):
    # x^T pre-tiled: xt[p, jp, k, s'] = x[jp*512+s', k*128+p]
    xts = []
    for b in range(B):
        xt = x[b].T.reshape(NDC, 128, NSB, SBK).transpose(1, 2, 0, 3)
        xts.append(np.ascontiguousarray(xt.astype(np_bf16)))
    in_maps = []
    for c in range(NCORES):
        b, hg = divmod(c, TP)
        sl = slice(hg * DP, (hg + 1) * DP)
        in_maps.append({
            "xt": xts[b],
            "wq": _pretile_qk(Wq[:, sl]),
            "wk": _pretile_qk(Wk[:, sl]),
            "wv": _pretile_k(Wv[:, sl]),
            "wo": _pretile_k(Wo[sl, :]),
            "bq": np.ascontiguousarray(bq[sl].reshape(NPC, 128).T),
            "bk": np.ascontiguousarray(bk[sl].reshape(NPC, 128).T),
            "bv": np.ascontiguousarray(bv[sl].reshape(1, DP)),
        })
    return in_maps


def run(inputs, **spmd_kwargs):
    """Build, run on 8 cores, gather. Returns (output, BassKernelResults)."""
    args = {k: np.asarray(v, dtype=np.float32) for k, v in inputs.items()}
    nc = _get_program()
    in_maps = _make_in_maps(
        args["x"], args["Wq"], args["bq"], args["Wk"], args["bk"],
        args["Wv"], args["bv"], args["Wo"], args["bo"],
    )
    res = run_bass_kernel_spmd(nc, in_maps, list(range(NCORES)), **spmd_kwargs)
    out = np.zeros((B, S, D), dtype=np.float32)
    for c in range(NCORES):
        b = c // TP
        out[b] += res.results[c]["out"].astype(np.float32)
    out += args["bo"]
    return out, res


def kernel(**inputs):
    out, _ = run(inputs)
    return out


# revision 39
# speedup vs baseline: 1.0215x; 1.0185x over previous
"""Multi-head attention (B=2, S=2048, D=1024, H=16, Dk=64) on 8 NeuronCores.

Sharding: 2-way data parallel over batch x 4-way tensor parallel over heads.
Core c handles batch c//4 and heads (c%4)*4 .. (c%4)*4+3, i.e. a 256-column
slice of the QKV projections and the matching 256-row slice of Wo. Each core
computes a partial output projection [S, D] in bf16; the host sums the 4
partials per batch in fp32 (the all-reduce of the sharding hint) and stacks
the batches.

All matmul operands are bf16 (PE full rate + fast weight load; fp32/f32r
matmuls run in multi-pass fp32_mode=HIGH at ~1/3 rate), accumulation is
always fp32 in PSUM. x is transposed AND pre-tiled on the host into the
exact SBUF layouts ([partition, chunk, free], j-block major for x^T) so
every DMA is a contiguous per-partition slab - descriptor generation cost
on the issuing engine is negligible and the first K projection starts after
~1MB of transfer.

On-core algorithm:
  Q^T, K^T head-packed [128, 2, S] (head parity on partition halves 0-63/
  64-127 so the two heads' K=64 score matmuls run concurrently in separate
  PE row groups) and V in natural [t, d'] layout augmented with a ones
  column -> S^T = K_h Q_h^T -> exp (1/8 scale folded in; no max subtraction:
  scores are O(5)) -> C^T = V_aug^T @ expS^T where the ones row yields the
  softmax denominator for free -> normalize -> partial out = C^T.T @ Wo.

The exp stream is split between the ACT engine (exact table exp) and the
otherwise-idle DVE via a one-instruction Schraudolph exp in bf16 bit space
(scores -> affine -> int16 round -> bitcast bf16), used on a minority of
t-tiles so the added rms error stays well under the tolerance.

Emission interleaves the projections and output-projection drains into the
attention t-loop so the PE never idles (HAM stays warm) while ACT/DVE grind
through the exp stream.
"""
from collections import defaultdict
from contextlib import ExitStack

import numpy as np
import ml_dtypes
import concourse.bass as bass
import concourse.mybir as mybir
import concourse.tile as tile
from concourse import bacc
from concourse.bass_utils import run_bass_kernel_spmd
from concourse.masks import make_identity

f32 = mybir.dt.float32
bf16 = mybir.dt.bfloat16
fp8 = mybir.dt.float8e4
i16 = mybir.dt.int16
DR = mybir.MatmulPerfMode.DoubleRow
AF = mybir.ActivationFunctionType
ALU = mybir.AluOpType
np_bf16 = ml_dtypes.bfloat16

B, S, D = 2, 2048, 1024
H, DK = 16, 64
NCORES = 8
TP = 4                 # tensor-parallel factor (head groups)
HPC = H // TP          # 4 heads per core
DP = HPC * DK          # 256 = per-core d' slice
SBK = 512              # s-block for attention streaming
NSB = S // SBK         # 4
NT = S // 128          # 16 t-tiles
NDC = D // 128         # 8 contraction chunks over D
NPC = DP // 128        # 2 chunks over d'

# Schraudolph fast-exp in bf16 bit space: bits(2^g) = round(2^7*(g+127-c)),
# g = s * log2e/8; c tuned for min rms of the linear-mantissa approximation.
SCH_C = 0.0580
SCH_A = float(2.0**7 * np.log2(np.e) / 8.0)
SCH_B = float(2.0**7 * (127.0 - 4.0 - SCH_C))  # extra -4: exp values scaled
# by 1/16 (cancels in the softmax ratio) so the fp8 tiles stay in range
ESC = float(-4.0 * np.log(2.0))               # ACT bias: exp(s/8 - 4ln2)

# (block, t) tiles whose exp runs on the DVE instead of ACT; chosen as
# aligned t-pairs because ACT pairs feed DoubleRow-fp8 PV matmuls while
# DVE pairs keep plain bf16 PV.
DVE_TILES = frozenset(
    (b, t) for b in range(2, 8) for t in (6, 7, 12, 13)
)

_prog_cache = {}


def _build_program():
    nc = bacc.Bacc()
    # dram layouts == sbuf tile layouts (host pre-tiles): contiguous DMAs
    xt = nc.dram_tensor("xt", [128, NSB, NDC, SBK], bf16, kind="ExternalInput")
    wq = nc.dram_tensor("wq", [128, NPC, NDC, 128], bf16, kind="ExternalInput")
    wk = nc.dram_tensor("wk", [128, NPC, NDC, 128], bf16, kind="ExternalInput")
    wv = nc.dram_tensor("wv", [128, NDC, DP], bf16, kind="ExternalInput")
    wo = nc.dram_tensor("wo", [128, NPC, D], bf16, kind="ExternalInput")
    bq = nc.dram_tensor("bq", [128, NPC], f32, kind="ExternalInput")
    bk = nc.dram_tensor("bk", [128, NPC], f32, kind="ExternalInput")
    bv = nc.dram_tensor("bv", [1, DP], f32, kind="ExternalInput")
    out = nc.dram_tensor("out", [S, D], bf16, kind="ExternalOutput")

    with tile.TileContext(nc) as tc, ExitStack() as top:
        const = top.enter_context(tc.tile_pool(name="const", bufs=1))
        big = top.enter_context(tc.tile_pool(name="big", bufs=1))
        esp = top.enter_context(tc.tile_pool(name="esp", bufs=5))
        smal = top.enter_context(tc.tile_pool(name="smal", bufs=2))
        outp = top.enter_context(tc.tile_pool(name="outp", bufs=2))
        ps_s = top.enter_context(tc.tile_pool(name="ps_s", bufs=2, space="PSUM"))
        ps_x = top.enter_context(tc.tile_pool(name="ps_x", bufs=2, space="PSUM"))
        ps_c = top.enter_context(tc.tile_pool(name="ps_c", bufs=1, space="PSUM"))

        # persistent activations
        xt_r = big.tile([128, NSB, NDC, SBK], bf16)
        qt_r = big.tile([128, NPC, S], bf16)
        kt_r = big.tile([128, NPC, S], bf16)
        vaug = big.tile([128, NT, HPC, DK + 1], bf16)
        ct_r = big.tile([128, NPC, S], bf16)

        wq_r = const.tile([128, NPC, NDC, 128], bf16)
        wk_r = const.tile([128, NPC, NDC, 128], bf16)
        wv_r = const.tile([128, NDC, DP], bf16)
        wo_r = const.tile([128, NPC, D], bf16)
        bq_sb = const.tile([128, NPC], f32)
        bk_sb = const.tile([128, NPC], f32)
        bv_b = const.tile([128, DP], f32)

        # ---- loads: x^T j'-blocks on the sync HWDGE queue (the first K
        # projection starts after ~1MB), weights on the gpsimd SWDGE
        # queues (wk first: it gates the first matmul) ----
        # biases first on the scalar HWDGE queue (tiny; bv_1 must land
        # early so the gpsimd broadcast at the END of its queue never gates
        # the weight loads), then the second half of the first x^T block
        nc.scalar.dma_start(out=bk_sb, in_=bk[:, :])
        nc.scalar.dma_start(out=bq_sb, in_=bq[:, :])
        bv_1 = const.tile([1, DP], f32)
        nc.scalar.dma_start(out=bv_1, in_=bv[:, :])
        nc.sync.dma_start(out=xt_r[:, 0, 0:4], in_=xt[:, 0, 0:4])
        nc.scalar.dma_start(out=xt_r[:, 0, 4:8], in_=xt[:, 0, 4:8])
        for jp in range(1, NSB):
            nc.sync.dma_start(out=xt_r[:, jp], in_=xt[:, jp])
        # c=0 halves of wk/wq first (they gate the first matmuls), then
        # wv (first V rides the pre-attention gap), then the rest; the
        # bv broadcast (needed by V(0)'s drain, ~20us in) goes last
        nc.gpsimd.dma_start(out=wk_r[:, 0], in_=wk[:, 0])
        nc.gpsimd.dma_start(out=wq_r[:, 0], in_=wq[:, 0])
        nc.gpsimd.dma_start(out=wv_r, in_=wv[:, :, :])
        nc.gpsimd.dma_start(out=wk_r[:, 1], in_=wk[:, 1])
        nc.gpsimd.dma_start(out=wq_r[:, 1], in_=wq[:, 1])
        nc.gpsimd.dma_start(out=wo_r, in_=wo[:, :, :])
        nc.gpsimd.partition_broadcast(bv_b, bv_1)
        ones_f = const.tile([128, NT, HPC], bf16)
        nc.vector.memset(ones_f, 1.0)
        esc_b = const.tile([128, 1], f32)
        nc.vector.memset(esc_b, ESC)
        ident = const.tile([128, 128], f32)
        make_identity(nc, ident)
        ones64 = const.tile([1, 64], f32)
        nc.vector.memset(ones64, 1.0)
        nc.vector.tensor_copy(out=vaug[:, :, :, DK], in_=ones_f)

        # ---- projection helpers ----
        # Q/K projections are emitted in two 4-matmul halves at consecutive
        # steps so no single step carries a ~1.7us PE lump (ACT can only
        # buffer ~1 score tile ahead; any lump starves the exp stream).
        proj_halves = {}

        def proj_qk_half(wr, bias_sb, dst, c, j, half):
            key = (id(wr), c, j)
            if half == 0:
                proj_halves[key] = ps_x.tile(
                    [128, SBK], f32, tag="px", bufs=2,
                    name=f"pj{id(wr)%97}_{c}_{j}")
            pq = proj_halves[key]
            for k in range(4 * half, 4 * half + 4):
                nc.tensor.matmul(
                    out=pq,
                    lhsT=wr[:, c, k, :],
                    rhs=xt_r[:, j, k, :],
                    start=(k == 0), stop=(k == NDC - 1),
                )
            if half == 1:
                nc.vector.tensor_scalar_add(
                    out=dst[:, c, j * SBK:(j + 1) * SBK],
                    in0=pq, scalar1=bias_sb[:, c:c + 1],
                )

        def proj_qk(wr, bias_sb, dst, c, j):
            proj_qk_half(wr, bias_sb, dst, c, j, 0)
            proj_qk_half(wr, bias_sb, dst, c, j, 1)

        def proj_v(st):
            pv = ps_x.tile([128, DP], f32, tag="px", bufs=2, name=f"pv{st}")
            for k in range(NDC):
                nc.tensor.matmul(
                    out=pv,
                    lhsT=xt_r[:, st // 4, k, (st % 4) * 128:(st % 4 + 1) * 128],
                    rhs=wv_r[:, k, :],
                    start=(k == 0), stop=(k == NDC - 1),
                )
            nc.vector.tensor_add(
                out=vaug[:, st, :, 0:DK],
                in0=pv.rearrange("p (h d) -> p h d", h=HPC),
                in1=bv_b.rearrange("p (h d) -> p h d", h=HPC),
            )

        drain_flip = [0]

        def outproj_result(st, nh, use_ss=False):
            # tail results borrow the score-psum banks (idle after the last
            # exp) so four results pipeline instead of two
            if use_ss:
                po = ps_s.tile([128, 512], f32, tag="ss", bufs=2,
                               name=f"po{st}_{nh}")
            else:
                po = ps_x.tile([128, 512], f32, tag="px", bufs=2,
                               name=f"po{st}_{nh}")
            for c in range(NPC):
                nc.tensor.matmul(
                    out=po,
                    lhsT=ct_r[:, c, st * 128:(st + 1) * 128],
                    rhs=wo_r[:, c, nh * 512:(nh + 1) * 512],
                    start=(c == 0), stop=(c == NPC - 1),
                )
            ob = outp.tile([128, 512], bf16, tag="ob", name=f"ob{st}_{nh}")
            drain_flip[0] ^= 1
            if drain_flip[0]:
                nc.vector.tensor_copy(out=ob, in_=po)
            else:
                nc.scalar.copy(out=ob, in_=po)
            nc.sync.dma_start(
                out=out[st * 128:(st + 1) * 128, nh * 512:(nh + 1) * 512],
                in_=ob,
            )

        # normalize runs in three stages emitted ~2 steps apart so no
        # DVE/gpsimd op is ever enqueued before its cross-engine dependency
        # is already satisfied (head-of-line blocking stalled the whole
        # pipeline at every block boundary otherwise).
        norm_state = {}

        def norm_stage1(j, hp, pcs):
            cus = []
            for hh in range(2):
                cu = smal.tile([DK + 1, SBK], f32, tag=f"cu{hh}", bufs=1,
                               name=f"cu{j}{hp}{hh}")
                if hh == 0:
                    nc.scalar.copy(out=cu, in_=pcs[hh])
                else:
                    nc.vector.tensor_copy(out=cu, in_=pcs[hh])
                cus.append(cu)
            # denominator rows [1,512] -> partition-major [128,8] via eight
            # skinny PE transposes (the gpsimd gather DMA costs ~4us in
            # 4-byte descriptors; the PE does it in ~1us)
            dn_ps = ps_x.tile([128, 8], f32, tag="px", bufs=2,
                              name=f"dnp{j}{hp}")
            for hh in range(2):
                for cc in range(4):
                    nc.tensor.transpose(
                        out=dn_ps[:, hh * 4 + cc:hh * 4 + cc + 1],
                        in_=cus[hh][DK:DK + 1, cc * 128:(cc + 1) * 128],
                        identity=ident[DK:DK + 1, DK:DK + 1],
                    )
            dnT = smal.tile([128, 8], f32, tag="dnT", name=f"dnT{j}{hp}")
            nc.vector.tensor_copy(out=dnT, in_=dn_ps)
            norm_state[(j, hp)] = [cus, dnT, None]

        def norm_stage2(j, hp):
            cus, dnT, _ = norm_state[(j, hp)]
            rT = smal.tile([128, 8], f32, tag="rT", name=f"rT{j}{hp}")
            nc.vector.reciprocal(out=rT, in_=dnT)
            # back to partition-0 rows (rhs of a matmul must be 0/32/64/96
            # aligned) via skinny transposes, then broadcast down 64
            # partitions per head half with ones-column K=1 matmuls
            rf = smal.tile([1, 2, SBK], f32, tag="rf", name=f"rf{j}{hp}")
            for hh in range(2):
                rfp = ps_x.tile([1, SBK], f32, tag="px", bufs=2,
                                name=f"rfp{j}{hp}{hh}")
                for cc in range(4):
                    nc.tensor.transpose(
                        out=rfp[0:1, cc * 128:(cc + 1) * 128],
                        in_=rT[:, hh * 4 + cc:hh * 4 + cc + 1],
                        identity=ident,
                    )
                nc.vector.tensor_copy(out=rf[:, hh, :], in_=rfp)
            rb_ps = ps_x.tile([128, SBK], f32, tag="px", bufs=2,
                              name=f"rbp{j}{hp}")
            for hh in range(2):
                for cc in range(4):
                    nc.tensor.matmul(
                        out=rb_ps[hh * 64:(hh + 1) * 64,
                                  cc * 128:(cc + 1) * 128],
                        lhsT=ones64,
                        rhs=rf[0:1, hh, cc * 128:(cc + 1) * 128],
                        start=True, stop=True,
                        tile_position=(0, hh * 64),
                    )
            norm_state[(j, hp)][2] = rb_ps

        def norm_stage3(j, hp):
            cus, _, rb_ps = norm_state[(j, hp)]
            for hh in range(2):
                nc.vector.tensor_mul(
                    out=ct_r[hh * 64:(hh + 1) * 64, hp, j * SBK:(j + 1) * SBK],
                    in0=cus[hh][0:DK, :],
                    in1=rb_ps[hh * 64:(hh + 1) * 64, :],
                )

        TAIL_OPS = []

        # ---- static filler schedule: (block, t) -> list of closures ----
        # MAND runs before the step's score matmul (front=True prepends:
        # projection halves must free their psum bank before V/outproj
        # allocate it); POST runs after the step's exp is emitted.
        # Blocks run hp-major: B0..B3 = (j,0), B4..B7 = (j,1). The c=1
        # projections aren't needed until B4, so the early blocks stay light
        # and the exp stream paces the kernel from B1 on.
        BLOCKS = [(j, 0) for j in range(NSB)] + [(j, 1) for j in range(NSB)]
        MAND = defaultdict(list)

        POST = defaultdict(list)

        def sched(b, t, fn, front=False):
            if front:
                MAND[(b, t)].insert(0, fn)
            else:
                MAND[(b, t)].append(fn)

        def sched_proj(b, t0, wr, bias_sb, dst, c, j):
            sched(b, t0, (lambda: proj_qk_half(wr, bias_sb, dst, c, j, 0)),
                  front=True)
            sched(b, t0 + 1, (lambda: proj_qk_half(wr, bias_sb, dst, c, j, 1)),
                  front=True)

        # B0 ((j0,hp0)): V projections ride post-step (V(st) emitted right
        # after step st's exp, consumed by the PV pair drained entering step
        # st+3; never ahead of the score matmuls, so the exp stream starts
        # as soon as the first K/Q chunks land), remaining K(c0) chunks land
        # just before their t-tiles need them.
        for st in range(NT):
            POST[(0, st)].append((lambda st=st: proj_v(st)))
        sched_proj(0, 1, wk_r, bk_sb, kt_r, 0, 1)
        sched_proj(0, 4, wk_r, bk_sb, kt_r, 0, 2)
        sched_proj(0, 8, wk_r, bk_sb, kt_r, 0, 3)
        sched_proj(0, 11, wq_r, bq_sb, qt_r, 0, 1)
        # B1..B3: spread K(c1) and the remaining Q projections evenly
        sched_proj(1, 4, wk_r, bk_sb, kt_r, 1, 0)
        sched_proj(1, 11, wq_r, bq_sb, qt_r, 0, 2)
        sched_proj(2, 1, wk_r, bk_sb, kt_r, 1, 1)
        sched_proj(2, 4, wk_r, bk_sb, kt_r, 1, 2)
        sched_proj(2, 11, wq_r, bq_sb, qt_r, 0, 3)
        sched_proj(3, 1, wk_r, bk_sb, kt_r, 1, 3)
        sched_proj(3, 4, wq_r, bq_sb, qt_r, 1, 0)
        sched_proj(3, 11, wq_r, bq_sb, qt_r, 1, 1)
        sched_proj(4, 8, wq_r, bq_sb, qt_r, 1, 2)
        sched_proj(5, 8, wq_r, bq_sb, qt_r, 1, 3)
        # normalize stages 2/3 of block b land early in block b+1
        for b in range(2 * NSB - 1):
            j, hp = BLOCKS[b]
            sched(b + 1, 3, (lambda j=j, hp=hp: norm_stage2(j, hp)))
            sched(b + 1, 4, (lambda j=j, hp=hp: norm_stage3(j, hp)))
        # output projection of j needs ct from (j,0) AND (j,1); block (j,1)
        # is B4+j, its normalize finishes early in B5+j -> spread the eight
        # [128x512] results over B5+j / B6+j (j=2 spills 2, j=3 fully into
        # the tail).
        OP_SLOTS = [(0, 5), (0, 7), (0, 9), (0, 11), (0, 13), (0, 15),
                    (1, 1), (1, 3)]
        OP_SLOTS_LAST = [(0, 7), (0, 8), (0, 9), (0, 10),
                         (0, 11), (0, 12), (0, 13), (0, 14)]
        for j in (0, 1, 2):
            slots = OP_SLOTS_LAST if j == 2 else OP_SLOTS
            for i, (st, nh) in enumerate(
                    (st, nh)
                    for st in range(j * 4, (j + 1) * 4) for nh in range(2)):
                db, tt = slots[i]
                if 5 + j + db < 2 * NSB:
                    sched(5 + j + db, tt,
                          (lambda st=st, nh=nh: outproj_result(st, nh)))
                else:
                    TAIL_OPS.append((st, nh))

        # ---- attention driver: software-pipelined, drains t-pairs ----
        pend = []
        cur_es8 = [None]

        def drain_pv():
            j, hp, t0, es0, pcs = pend.pop(0)
            _, _, t1, es1, _ = pend.pop(0)
            for t, es in ((t0, es0), (t1, es1)):
                for hh in range(2):
                    nc.tensor.matmul(
                        out=pcs[hh],
                        lhsT=vaug[:, t, hp * 2 + hh, :],
                        rhs=es[:, hh, :],
                        start=(t == 0), stop=(t == NT - 1),
                    )
            if t1 == NT - 1:
                norm_stage1(j, hp, pcs)

        # pre-attention: K/Q for the first score matmul only
        proj_qk(wk_r, bk_sb, kt_r, 0, 0)
        proj_qk(wq_r, bq_sb, qt_r, 0, 0)

        pcs_by = {}
        for b in range(2 * NSB):
            j, hp = BLOCKS[b]
            pcs_by[(j, hp)] = [
                ps_c.tile([DK + 1, SBK], f32, tag=f"pc{hh}", name=f"pc{hh}_{j}_{hp}")
                for hh in range(2)]
            for t in range(NT):
                if len(pend) >= 4 or (t == 1 and len(pend) >= 2):
                    drain_pv()
                for fn in MAND[(b, t)]:
                    fn()
                ss = ps_s.tile([128, 2, SBK], f32, tag="ss", name=f"ss{b}_{t}")
                for hh in range(2):
                    nc.tensor.matmul(
                        out=ss[:, hh, :],
                        lhsT=kt_r[hh * 64:(hh + 1) * 64, hp, t * 128:(t + 1) * 128],
                        rhs=qt_r[hh * 64:(hh + 1) * 64, hp, j * SBK:(j + 1) * SBK],
                        start=True, stop=True,
                    )
                if (b, t) in DVE_TILES:
                    # Schraudolph: bf16 bits of exp(s/8)/16 = int16(A*s + B);
                    # the PV matmul reads the int16 tile as bf16 directly.
                    esi = esp.tile([128, 2, SBK], i16, tag="es", name=f"esi{b}_{t}")
                    nc.vector.tensor_scalar(
                        out=esi, in0=ss,
                        scalar1=SCH_A, scalar2=SCH_B,
                        op0=ALU.mult, op1=ALU.add,
                    )
                    es = esi.bitcast(bf16)
                else:
                    es = esp.tile([128, 2, SBK], bf16, tag="es", name=f"es{b}_{t}")
                    nc.scalar.activation(out=es, in_=ss, func=AF.Exp,
                                         scale=0.125, bias=esc_b[:, 0:1])
                pend.append((j, hp, t, es, pcs_by[(j, hp)]))
                for fn in POST[(b, t)]:
                    fn()
        drain_pv()
        drain_pv()
        # tail: finish the last block's normalize, then the remaining
        # output-projection results
        norm_stage2(3, 1)
        norm_stage3(3, 1)
        for st in range(12, 16):
            for nh in range(2):
                TAIL_OPS.append((st, nh))
        for i, (st, nh) in enumerate(TAIL_OPS):
            outproj_result(st, nh, use_ss=(i % 2 == 1))

    nc.finalize()
    return nc


def _get_program():
    if "nc" not in _prog_cache:
        _prog_cache["nc"] = _build_program()
    return _prog_cache["nc"]


def _pretile_k(w):
    """[D, d'] fp32 -> [128, D//128, d'] bf16 (partition-major chunks)."""
    dp = w.shape[1]
    return np.ascontiguousarray(
        w.reshape(-1, 128, dp).transpose(1, 0, 2).astype(np_bf16))


def _pretile_qk(w):
    """[D, 256] fp32 -> [128, 2, D//128, 128] bf16 (c-major halves)."""
    return np.ascontiguousarray(
        w.reshape(NDC, 128, NPC, 128).transpose(1, 2, 0, 3).astype(np_bf16))


def _make_in_maps(x, Wq, bq, Wk, bk, Wv, bv, Wo, bo):
    # x^T pre-tiled: xt[p, jp, k, s'] = x[jp*512+s', k*128+p]
    xts = []
    for b in range(B):
        xt = x[b].T.reshape(NDC, 128, NSB, SBK).transpose(1, 2, 0, 3)
        xts.append(np.ascontiguousarray(xt.astype(np_bf16)))
    in_maps = []
    for c in range(NCORES):
        b, hg = divmod(c, TP)
        sl = slice(hg * DP, (hg + 1) * DP)
        in_maps.append({
            "xt": xts[b],
            "wq": _pretile_qk(Wq[:, sl]),
            "wk": _pretile_qk(Wk[:, sl]),
            "wv": _pretile_k(Wv[:, sl]),
            "wo": _pretile_k(Wo[sl, :]),
            "bq": np.ascontiguousarray(bq[sl].reshape(NPC, 128).T),
            "bk": np.ascontiguousarray(bk[sl].reshape(NPC, 128).T),
            "bv": np.ascontiguousarray(bv[sl].reshape(1, DP)),
        })
    return in_maps


def run(inputs, **spmd_kwargs):
    """Build, run on 8 cores, gather. Returns (output, BassKernelResults)."""
    args = {k: np.asarray(v, dtype=np.float32) for k, v in inputs.items()}
    nc = _get_program()
    in_maps = _make_in_maps(
        args["x"], args["Wq"], args["bq"], args["Wk"], args["bk"],
        args["Wv"], args["bv"], args["Wo"], args["bo"],
    )
    res = run_bass_kernel_spmd(nc, in_maps, list(range(NCORES)), **spmd_kwargs)
    out = np.zeros((B, S, D), dtype=np.float32)
    for c in range(NCORES):
        b = c // TP
        out[b] += res.results[c]["out"].astype(np.float32)
    out += args["bo"]
    return out, res


def kernel(**inputs):
    out, _ = run(inputs)
    return out
